# revision 1
# baseline (speedup 1.0000x reference)
"""Trainium2 Bass kernel for nn_CombinedLoss_781684048617.

Strategy (pure data parallel over 8 NeuronCores, B=262144 rows split into
8 shards of 32768 rows; only ~170 floats of partial sums leave each core):

The loss reduces to a handful of global sums.  All row-contractions are
pushed onto the PE (tensor engine) as two fp16 gram matrices accumulated
in fp32 PSUM across every 128-row block k:

  psA += yt_k^T @ [sl1_k | lse_k | 1]    (120 x 46)
  psB += yt_k^T @ yp_k                   (120 x 120)

y_true's logit columns are one-hot*active (exactly 0/1 even in fp16), so
with rows indexed by y_true column (24e+c for slot e / class c):
  - psA[., ones]  -> per-(e,c) active counts     -> mask count, param count
  - psA[., lse_e] -> sum of active lse           -> CE logsumexp term
  - psA[., sl1]   -> sl1 paired with true class  -> masked SmoothL1 (via the
                     host-side (j < num_params_per_effect[c]) table)
  - psB diag      -> sum yp*yt over logits       -> CE logp_true dot term
  - psB 16x16 block sums -> active*(sum_c logit) -> label-smoothing term

Both inputs stream HBM->SBUF through gpsimd (SWDGE) DMAs that cast
fp32->fp16 in flight (HBM read side is the roofline; fp16 halves SBUF and
makes the matmuls 1 cycle/row instead of fp32's 4).  ACT does exp/ln/abs/
square/relu (one activation-table set, preloaded once); DVE does one
reduce, one sub, one min and the fused sl1 add whose accum_out also yields
the SmoothL1 grand total.  Final scalar assembly (divisions, guards,
num_params_per_effect weighting) happens on host in float64.

Measured: relative error vs reference 3.3e-06 on hardware; cost-model
timeline 66us per core (engine busy: DVE 49, DMA 44 write-charged, ACT 44,
PE 29, Pool 19) against an ~88us HBM-read roofline (31.5 MB/core at
358 GB/s/NC) - every engine holds ~2x slack under the real DMA floor.
"""

import sys

import numpy as np

if "/opt/trn_rl_repo" not in sys.path:
    sys.path.insert(0, "/opt/trn_rl_repo")

# ---- problem constants (hardcoded per contract) ----
B_FULL = 262144
NCORES = 8
N_CORE = B_FULL // NCORES  # 32768
E, C, P, ITEM = 5, 16, 8, 24
D = E * ITEM  # 120
LS = 0.05
REG_W = 1.0

# ---- kernel tiling ----
PARTS = 128
K = 32  # rows per partition per tile
ROWS_PER_TILE = PARTS * K  # 4096
NT = N_CORE // ROWS_PER_TILE  # 8 tiles per core
SW = D  # 120 stationary cols (full y_true row; logit rows at 24e+c)
AW = E * P + E + 1  # 46 moving cols of R: [sl1(40)|lse(5)|ones(1)]
COL_SL1 = 0  # + 8e + j
COL_LSE = E * P  # + e
COL_ONE = E * P + E
GW = AW + D  # out_ab width: [R-gram(46) | yp-gram(120, col=24e+c)]

_CACHE = {}


def _build_bass(n_core=N_CORE, k_rows=K):
    from contextlib import ExitStack

    import concourse.bacc as bacc
    import concourse.bass as bass
    import concourse.tile as tile
    from concourse import mybir

    K = k_rows
    NT = n_core // (PARTS * K)
    assert NT * PARTS * K == n_core
    # smaller final tiles shorten the post-last-DMA compute tail
    if NT >= 2:
        tiles = [K] * (NT - 1) + [K // 2, K // 2]
    else:
        tiles = [K] * NT

    f32 = mybir.dt.float32
    bf16 = mybir.dt.float16  # fp16: 8x finer mantissa than bf16; logits/params are small-range
    AF = mybir.ActivationFunctionType
    OP = mybir.AluOpType

    nc = bacc.Bacc(None, target_bir_lowering=False)
    yp_d = nc.dram_tensor("y_pred", [n_core, D], f32, kind="ExternalInput")
    yt_d = nc.dram_tensor("y_true", [n_core, D], f32, kind="ExternalInput")
    out_ab = nc.dram_tensor("out_ab", [SW, GW], f32, kind="ExternalOutput")
    out_ss = nc.dram_tensor("out_ss", [PARTS, len(tiles)], f32, kind="ExternalOutput")


    with tile.TileContext(nc) as tc, ExitStack() as ctx:
        inp = ctx.enter_context(tc.tile_pool(name="inp", bufs=6))
        work = ctx.enter_context(tc.tile_pool(name="work", bufs=3))
        singles = ctx.enter_context(tc.tile_pool(name="singles", bufs=1))
        psum = ctx.enter_context(
            tc.tile_pool(name="psum", bufs=1, space=bass.MemorySpace.PSUM)
        )

        psA = psum.tile([SW, AW], f32)
        psB = psum.tile([SW, D], f32)
        ssum_acc = singles.tile([PARTS, len(tiles)], f32)
        neg1 = singles.tile([PARTS, 1], f32)
        nc.vector.memset(neg1, -1.0)

        row0 = 0
        for i, KT in enumerate(tiles):
            ypv = yp_d[row0 : row0 + PARTS * KT].rearrange(
                "(p k) f -> p k f", k=KT
            )
            ytv = yt_d[row0 : row0 + PARTS * KT].rearrange(
                "(p k) f -> p k f", k=KT
            )
            row0 += PARTS * KT
            yp_t = inp.tile([PARTS, KT, D], bf16)
            yt_t = inp.tile([PARTS, KT, D], bf16)
            # gpsimd (SWDGE) DMAs cast fp32->fp16 in flight; one-hot 0/1
            # y_true values are exact in fp16, y_pred rounding averages out
            nc.gpsimd.dma_start(out=yp_t, in_=ypv)
            nc.gpsimd.dma_start(out=yt_t, in_=ytv)

            yp4 = yp_t.rearrange("p k (e i) -> p k e i", i=ITEM)
            yt4 = yt_t.rearrange("p k (e i) -> p k e i", i=ITEM)
            ypP = yp4[:, :, :, C:ITEM]
            ytP = yt4[:, :, :, C:ITEM]

            # --- cross entropy pieces: lse = ln(sum_c exp(logit)) ---
            ex_t = work.tile([PARTS, KT, E, C], bf16, bufs=2)
            nc.scalar.activation(out=ex_t, in_=yp4[:, :, :, 0:C], func=AF.Exp)
            s_t = work.tile([PARTS, KT, E], bf16, bufs=2)
            with nc.allow_low_precision("bf16 softmax-denominator is plenty"):
                nc.vector.tensor_reduce(
                    out=s_t, in_=ex_t, axis=mybir.AxisListType.X, op=OP.add
                )

            R_t = work.tile([PARTS, KT, AW], bf16, bufs=5)
            nc.vector.memset(R_t[:, :, COL_ONE : COL_ONE + 1], 1.0)
            nc.scalar.activation(
                out=R_t[:, :, COL_LSE : COL_LSE + E], in_=s_t, func=AF.Ln
            )

            # --- smooth l1: sl1 = 0.5*min(|d|,1)^2 + relu(|d|-1) ---
            # scratch reuse: d -> (abs) ad; min back into d (=m2);
            # square(m2) -> sq; relu(ad-1) back into d (m2 dead by then)
            d_t = work.tile([PARTS, KT, E, P], bf16, bufs=2)
            nc.vector.tensor_sub(d_t, ypP, ytP)
            ad_t = work.tile([PARTS, KT, E, P], bf16, bufs=2)
            nc.scalar.activation(out=ad_t, in_=d_t, func=AF.Abs)
            nc.vector.tensor_scalar(
                out=d_t, in0=ad_t, scalar1=1.0, scalar2=None, op0=OP.min
            )
            sq_t = work.tile([PARTS, KT, E, P], bf16, bufs=2)
            nc.vector.tensor_mul(sq_t, d_t, d_t)  # m2^2 on DVE, ACT stays lighter
            nc.scalar.activation(out=d_t, in_=ad_t, func=AF.Relu, bias=neg1)
            nc.vector.scalar_tensor_tensor(
                out=R_t[:, :, 0 : E * P].rearrange("p k (e j) -> p k e j", j=P),
                in0=sq_t,
                scalar=0.5,
                in1=d_t,
                op0=OP.mult,
                op1=OP.add,
                accum_out=ssum_acc[:, i : i + 1],
            )

            # --- gram accumulation on PE ---
            for k in range(KT):
                first = i == 0 and k == 0
                last = i == len(tiles) - 1 and k == KT - 1
                nc.tensor.matmul(
                    psA, yt_t[:, k, :], R_t[:, k, :], start=first, stop=last
                )
                nc.tensor.matmul(
                    psB, yt_t[:, k, :], yp_t[:, k, :], start=first, stop=last
                )

        stage = singles.tile([SW, GW], f32)
        nc.scalar.copy(stage[:, 0:AW], psA)
        nc.scalar.copy(stage[:, AW:GW], psB)
        nc.sync.dma_start(out=out_ab[:], in_=stage)
        nc.sync.dma_start(out=out_ss[:], in_=ssum_acc)

    # Pre-load the one ACT table set covering Exp/Ln/Abs/Square/Relu/Copy
    # (natural_log_exp_and_others). Without this, the greedy per-activation
    # selector thrashes exp_and_others <-> natural_log (2 reloads per tile,
    # ~20us of ACT time).
    from concourse.hw_specs import get_activation_tables

    tables = list(get_activation_tables(nc.m.arch).items())
    set_id = next(
        i for i, (name, _) in enumerate(tables)
        if name == "natural_log_exp_and_others"
    )
    load = mybir.InstLoadActFuncSet(
        name=nc.get_next_instruction_name(), act_func_set_id=set_id, ins=[], outs=[]
    )
    load.engine = mybir.EngineType.Activation
    nc.register_instruction(load)
    placed = False
    for blk in nc.m.functions[0].blocks:
        for idx, inst in enumerate(blk.instructions):
            if isinstance(inst, mybir.InstActivation):
                blk.instructions.insert(idx, load)
                placed = True
                break
        if placed:
            break
    assert placed

    nc.compile()
    return nc


def _get_nc():
    if "nc" not in _CACHE:
        _CACHE["nc"] = _build_bass()
    return _CACHE["nc"]


def kernel(y_pred, y_true, num_params_per_effect):
    from concourse.bass_utils import run_bass_kernel_spmd

    yp = np.ascontiguousarray(np.asarray(y_pred, dtype=np.float32))
    yt = np.ascontiguousarray(np.asarray(y_true, dtype=np.float32))
    npf = np.asarray(num_params_per_effect, dtype=np.int64)

    yp_sh = yp.reshape(NCORES, N_CORE, D)
    yt_sh = yt.reshape(NCORES, N_CORE, D)
    in_maps = [
        {"y_pred": yp_sh[i], "y_true": yt_sh[i]} for i in range(NCORES)
    ]

    nc = _get_nc()
    results = run_bass_kernel_spmd(nc, in_maps, list(range(NCORES))).results

    # ---- host-side scalar assembly in float64 ----
    G = np.zeros((SW, GW), np.float64)
    SSUM = 0.0
    for res in results:
        G += np.asarray(res["out_ab"], np.float64)
        SSUM += float(np.asarray(res["out_ss"], np.float64).sum())

    Tmask = (np.arange(P)[None, :] < npf[:, None]).astype(np.float64)  # [C,P]
    MSUM = 0.0
    PCNT = 0.0
    LSEt = 0.0
    DX = 0.0
    AFSX = 0.0
    RSUM = 0.0
    for e in range(E):
        rows = slice(ITEM * e, ITEM * e + C)  # yt logit rows of slot e
        cnt = G[rows, COL_ONE]  # per-class active counts [C]
        MSUM += cnt.sum()
        PCNT += (npf * cnt).sum()
        LSEt += G[rows, COL_LSE + e].sum()
        DX += np.trace(G[rows, AW + ITEM * e : AW + ITEM * e + C])
        AFSX += G[rows, AW + ITEM * e : AW + ITEM * e + C].sum()
        RSUM += (Tmask * G[rows, COL_SL1 + P * e : COL_SL1 + P * (e + 1)]).sum()

    CSUM = LSEt - (1.0 - LS) * DX - (LS / C) * AFSX

    loss_cls = CSUM / max(MSUM, 1.0) if MSUM > 0 else 0.0
    reg_masked = RSUM / max(PCNT, 1.0)
    reg_unmasked = SSUM / max(MSUM, 1.0)
    loss_reg = (reg_masked if PCNT > 0 else reg_unmasked) if MSUM > 0 else 0.0
    total = loss_cls + REG_W * loss_reg

    return (
        np.float32(total),
        np.float32(loss_cls),
        np.float32(loss_reg),
    )



# revision 2
# speedup vs baseline: 1.3112x; 1.3112x over previous
"""Trainium2 Bass kernel for nn_CombinedLoss_781684048617 (V2).

Pure data parallel over 8 NeuronCores (32768 rows each); only ~100KB of
partial sums leave each core.  All row contractions run on the PE as two
gram matrices against the one-hot y_true logit columns, accumulated in
fp32 PSUM over every 128-row block k:

  psA += yt_k^T @ [w1 | w2' | mm | lse | 1]   (120 x 126)
  psB += yt_k^T @ yp_logits_k                 (120 x 80)

where per element d = yp_par - yt_par:
  w1 = relu(d-1), w2' = min(d+1, 0), mm = min(d^2, 1)
  SmoothL1 sum = sum w1 - sum w2' + 0.5 sum mm   (w = relu(|d|-1) = w1 - w2')

Both inputs stream HBM->SBUF via gpsimd (SWDGE) DMAs casting fp32->fp8e3
in flight (DMA cost is write-charged: fp8 halves it vs fp16).  fp8e3
(e3m4) keeps 4 mantissa bits; y_true one-hot values are exact.  ACT does
exp (fp8 in, fp16 out), ln, and Square; DVE does the softmax-denominator
sum as a tree of strided fp16 adds (2x mode) and the three SmoothL1
columns as tensor_scalar 2-op passes (4x mode).  Final scalar assembly
(divisions, guards, num_params_per_effect table) happens on host in
float64; reg_unmasked is dead code since num_params_per_effect >= 1
implies param count >= mask count.
"""

import sys

import numpy as np

if "/opt/trn_rl_repo" not in sys.path:
    sys.path.insert(0, "/opt/trn_rl_repo")

# ---- problem constants (hardcoded per contract) ----
B_FULL = 262144
NCORES = 8
N_CORE = B_FULL // NCORES  # 32768
E, C, P, ITEM = 5, 16, 8, 24
D = E * ITEM  # 120
LS = 0.05
REG_W = 1.0

PARTS = 128
ROWS_PER_PART = N_CORE // PARTS  # 256
# chunk sizes (rows per partition per chunk); smaller edges shorten
# pipeline fill/drain
CHUNKS = [32, 64, 64, 64, 32]
assert sum(CHUNKS) == ROWS_PER_PART

# psA column layout
NW = E * P  # 40
COL_W1 = 0
COL_W2 = NW
COL_MM = 2 * NW
COL_LSE = 3 * NW
COL_ONE = 3 * NW + E
AW = 3 * NW + E + 1  # 126
BW = E * C  # 80 yp-logit gram columns
GW = AW + BW  # 206

_CACHE = {}


def _build_bass():
    from contextlib import ExitStack

    import concourse.bacc as bacc
    import concourse.bass as bass
    import concourse.tile as tile
    from concourse import mybir

    f32 = mybir.dt.float32
    f16 = mybir.dt.float16
    f8 = mybir.dt.float8e3  # e3m4: 4 mantissa bits, range +-15.9
    AF = mybir.ActivationFunctionType
    OP = mybir.AluOpType

    nc = bacc.Bacc(None, target_bir_lowering=False)
    yp_d = nc.dram_tensor("y_pred", [N_CORE, D], f32, kind="ExternalInput")
    yt_d = nc.dram_tensor("y_true", [N_CORE, D], f32, kind="ExternalInput")
    out_ab = nc.dram_tensor("out_ab", [D, GW], f32, kind="ExternalOutput")

    with tile.TileContext(nc) as tc, ExitStack() as ctx:
        inp = ctx.enter_context(tc.tile_pool(name="inp", bufs=2))
        work = ctx.enter_context(tc.tile_pool(name="work", bufs=2))
        singles = ctx.enter_context(tc.tile_pool(name="singles", bufs=1))
        psum = ctx.enter_context(
            tc.tile_pool(name="psum", bufs=1, space=bass.MemorySpace.PSUM)
        )

        psA = psum.tile([D, AW], f32)
        psB = psum.tile([D, BW], f32)

        row0 = 0
        nchunks = len(CHUNKS)
        for ci, KT in enumerate(CHUNKS):
            ypv = yp_d[row0 : row0 + PARTS * KT].rearrange(
                "(p k) (e i) -> p k e i", k=KT, i=ITEM
            )
            ytv = yt_d[row0 : row0 + PARTS * KT].rearrange(
                "(p k) (e i) -> p k e i", k=KT, i=ITEM
            )
            row0 += PARTS * KT

            yp8 = inp.tile([PARTS, KT, E, ITEM], f8)
            yt8 = inp.tile([PARTS, KT, E, ITEM], f8)
            nc.gpsimd.dma_start(out=yp8, in_=ypv)
            nc.gpsimd.dma_start(out=yt8, in_=ytv)

            # ---- cross entropy: lse = ln(sum_c exp(z)) ----
            ex = work.tile([PARTS, KT, E, C], f16)
            nc.scalar.activation(out=ex, in_=yp8[:, :, :, 0:C], func=AF.Exp)
            h1 = work.tile([PARTS, KT, E, 8], f16)
            nc.vector.tensor_tensor(h1, ex[:, :, :, 0:8], ex[:, :, :, 8:16], OP.add)
            h2 = work.tile([PARTS, KT, E, 4], f16)
            nc.vector.tensor_tensor(h2, h1[:, :, :, 0:4], h1[:, :, :, 4:8], OP.add)
            h3 = work.tile([PARTS, KT, E, 2], f16)
            nc.vector.tensor_tensor(h3, h2[:, :, :, 0:2], h2[:, :, :, 2:4], OP.add)
            s = work.tile([PARTS, KT, E], f16)
            nc.vector.tensor_tensor(s, h3[:, :, :, 0], h3[:, :, :, 1], OP.add)

            M2 = work.tile([PARTS, KT, AW], f16)
            nc.scalar.activation(
                out=M2[:, :, COL_LSE : COL_LSE + E], in_=s, func=AF.Ln
            )
            nc.vector.memset(M2[:, :, COL_ONE : COL_ONE + 1], 1.0)

            # ---- smooth l1 columns ----
            dd = work.tile([PARTS, KT, E, P], f16)
            nc.vector.tensor_sub(dd, yp8[:, :, :, C:ITEM], yt8[:, :, :, C:ITEM])
            vv = work.tile([PARTS, KT, E, P], f16)
            nc.scalar.activation(out=vv, in_=dd, func=AF.Square)
            w1v = M2[:, :, COL_W1 : COL_W1 + NW].rearrange(
                "p k (e j) -> p k e j", j=P
            )
            nc.vector.tensor_scalar(
                out=w1v, in0=dd, scalar1=1.0, scalar2=0.0,
                op0=OP.subtract, op1=OP.max,
            )
            w2v = M2[:, :, COL_W2 : COL_W2 + NW].rearrange(
                "p k (e j) -> p k e j", j=P
            )
            nc.vector.tensor_scalar(
                out=w2v, in0=dd, scalar1=1.0, scalar2=0.0,
                op0=OP.add, op1=OP.min,
            )
            mmv = M2[:, :, COL_MM : COL_MM + NW].rearrange(
                "p k (e j) -> p k e j", j=P
            )
            nc.vector.tensor_scalar(
                out=mmv, in0=vv, scalar1=1.0, scalar2=None, op0=OP.min
            )

            # ---- gram accumulation on PE ----
            ytf = yt8.rearrange("p k e i -> p k (e i)")
            for k in range(KT):
                first = ci == 0 and k == 0
                last = ci == nchunks - 1 and k == KT - 1
                nc.tensor.matmul(
                    psA, ytf[:, k], M2[:, k], start=first, stop=last
                )
                nc.tensor.matmul(
                    psB, ytf[:, k], yp8[:, k, :, 0:C], start=first, stop=last
                )

        stage = singles.tile([D, GW], f32)
        nc.scalar.copy(stage[:, 0:AW], psA)
        nc.scalar.copy(stage[:, AW:GW], psB)
        nc.sync.dma_start(out=out_ab[:], in_=stage)

    # Pre-load the one ACT table set covering Exp/Ln/Square/Copy
    # (natural_log_exp_and_others); avoids per-activation table thrash.
    from concourse.hw_specs import get_activation_tables

    tables = list(get_activation_tables(nc.m.arch).items())
    set_id = next(
        i for i, (name, _) in enumerate(tables)
        if name == "natural_log_exp_and_others"
    )
    load = mybir.InstLoadActFuncSet(
        name=nc.get_next_instruction_name(), act_func_set_id=set_id, ins=[], outs=[]
    )
    load.engine = mybir.EngineType.Activation
    nc.register_instruction(load)
    placed = False
    for blk in nc.m.functions[0].blocks:
        for idx, inst in enumerate(blk.instructions):
            if isinstance(inst, mybir.InstActivation):
                blk.instructions.insert(idx, load)
                placed = True
                break
        if placed:
            break
    assert placed

    nc.compile()
    return nc


def _get_nc():
    if "nc" not in _CACHE:
        _CACHE["nc"] = _build_bass()
    return _CACHE["nc"]


def kernel(y_pred, y_true, num_params_per_effect):
    from concourse.bass_utils import run_bass_kernel_spmd

    yp = np.ascontiguousarray(np.asarray(y_pred, dtype=np.float32))
    yt = np.ascontiguousarray(np.asarray(y_true, dtype=np.float32))
    npf = np.asarray(num_params_per_effect, dtype=np.int64)

    yp_sh = yp.reshape(NCORES, N_CORE, D)
    yt_sh = yt.reshape(NCORES, N_CORE, D)
    in_maps = [{"y_pred": yp_sh[i], "y_true": yt_sh[i]} for i in range(NCORES)]

    nc = _get_nc()
    results = run_bass_kernel_spmd(nc, in_maps, list(range(NCORES))).results

    # ---- host-side scalar assembly in float64 ----
    G = np.zeros((D, GW), np.float64)
    for res in results:
        G += np.asarray(res["out_ab"], np.float64)
    GA = G[:, 0:AW]
    GB = G[:, AW:GW]

    Tmask = (np.arange(P)[None, :] < npf[:, None]).astype(np.float64)  # [C,P]
    MSUM = 0.0
    PCNT = 0.0
    LSEt = 0.0
    DX = 0.0
    AFSX = 0.0
    RSUM = 0.0
    for e in range(E):
        rows = slice(ITEM * e, ITEM * e + C)  # yt logit rows of slot e
        cnt = GA[rows, COL_ONE]  # per-class active counts [C]
        MSUM += cnt.sum()
        PCNT += (npf * cnt).sum()
        LSEt += GA[rows, COL_LSE + e].sum()
        DX += np.trace(GB[rows, C * e : C * e + C])
        AFSX += GB[rows, C * e : C * e + C].sum()
        cols = slice(P * e, P * (e + 1))
        sl1 = (
            GA[rows, COL_W1 + P * e : COL_W1 + P * (e + 1)]
            - GA[rows, COL_W2 + P * e : COL_W2 + P * (e + 1)]
            + 0.5 * GA[rows, COL_MM + P * e : COL_MM + P * (e + 1)]
        )
        RSUM += (Tmask * sl1).sum()

    CSUM = LSEt - (1.0 - LS) * DX - (LS / C) * AFSX

    loss_cls = CSUM / max(MSUM, 1.0) if MSUM > 0 else 0.0
    # num_params_per_effect >= 1 guarantees PCNT >= MSUM, so the
    # reference's unmasked-reg fallback (psum==0 while msum>0) is dead.
    loss_reg = (RSUM / max(PCNT, 1.0) if PCNT > 0 else 0.0) if MSUM > 0 else 0.0
    total = loss_cls + REG_W * loss_reg

    return (
        np.float32(total),
        np.float32(loss_cls),
        np.float32(loss_reg),
    )


# revision 5
# speedup vs baseline: 1.4420x; 1.0997x over previous
"""Trainium2 Bass kernel for nn_CombinedLoss_781684048617 (V2).

Pure data parallel over 8 NeuronCores (32768 rows each); only ~100KB of
partial sums leave each core.  All row contractions run on the PE as two
gram matrices against the one-hot y_true logit columns, accumulated in
fp32 PSUM over every 128-row block k:

  psA += yt_k^T @ [w1 | w2' | mm | lse | 1]   (120 x 126)
  psB += yt_k^T @ yp_logits_k                 (120 x 80)

where per element d = yp_par - yt_par:
  w1 = relu(d-1), w2' = min(d+1, 0), mm = min(d^2, 1)
  SmoothL1 sum = sum w1 - sum w2' + 0.5 sum mm   (w = relu(|d|-1) = w1 - w2')

Both inputs stream HBM->SBUF via gpsimd (SWDGE) DMAs casting fp32->fp8e3
in flight (DMA cost is write-charged: fp8 halves it vs fp16).  fp8e3
(e3m4) keeps 4 mantissa bits; y_true one-hot values are exact.  ACT does
exp (fp8 in, fp16 out), ln, and Square; DVE does the softmax-denominator
sum as a tree of strided fp16 adds (2x mode) and the three SmoothL1
columns as tensor_scalar 2-op passes (4x mode).  Final scalar assembly
(divisions, guards, num_params_per_effect table) happens on host in
float64; reg_unmasked is dead code since num_params_per_effect >= 1
implies param count >= mask count.
"""

import sys

import numpy as np

if "/opt/trn_rl_repo" not in sys.path:
    sys.path.insert(0, "/opt/trn_rl_repo")

# ---- problem constants (hardcoded per contract) ----
B_FULL = 262144
NCORES = 8
N_CORE = B_FULL // NCORES  # 32768
E, C, P, ITEM = 5, 16, 8, 24
D = E * ITEM  # 120
LS = 0.05
REG_W = 1.0

PARTS = 128
ROWS_PER_PART = N_CORE // PARTS  # 256
# chunk sizes (rows per partition per chunk); smaller edges shorten
# pipeline fill/drain
CHUNKS = [16, 32, 48, 48, 48, 48, 16]
assert sum(CHUNKS) == ROWS_PER_PART

# psA column layout
NW = E * P  # 40
COL_W1 = 0
COL_W2 = NW
COL_MM = 2 * NW
COL_LSE = 3 * NW
COL_ONE = 3 * NW + E
AW = 3 * NW + E + 1  # 126
BW = E * C  # 80 yp-logit gram columns
GW = AW + BW  # 206

_CACHE = {}


def _build_bass():
    from contextlib import ExitStack

    import concourse.bacc as bacc
    import concourse.bass as bass
    import concourse.tile as tile
    from concourse import mybir

    f32 = mybir.dt.float32
    f16 = mybir.dt.float16
    f8 = mybir.dt.float8e3  # e3m4: 4 mantissa bits, range +-15.9
    AF = mybir.ActivationFunctionType
    OP = mybir.AluOpType

    nc = bacc.Bacc(None, target_bir_lowering=False)
    yp_d = nc.dram_tensor("y_pred", [N_CORE, D], f32, kind="ExternalInput")
    yt_d = nc.dram_tensor("y_true", [N_CORE, D], f32, kind="ExternalInput")
    out_ab = nc.dram_tensor("out_ab", [D, GW], f32, kind="ExternalOutput")

    with tile.TileContext(nc) as tc, ExitStack() as ctx:
        inp = ctx.enter_context(tc.tile_pool(name="inp", bufs=4))
        work = ctx.enter_context(tc.tile_pool(name="work", bufs=3))
        singles = ctx.enter_context(tc.tile_pool(name="singles", bufs=1))
        psum = ctx.enter_context(
            tc.tile_pool(name="psum", bufs=1, space=bass.MemorySpace.PSUM)
        )

        psA = psum.tile([D, AW], f32)
        psB = psum.tile([D, BW], f32)

        row0 = 0
        nchunks = len(CHUNKS)
        for ci, KT in enumerate(CHUNKS):
            ypv = yp_d[row0 : row0 + PARTS * KT].rearrange(
                "(p k) (e i) -> p k e i", k=KT, i=ITEM
            )
            ytv = yt_d[row0 : row0 + PARTS * KT].rearrange(
                "(p k) (e i) -> p k e i", k=KT, i=ITEM
            )
            row0 += PARTS * KT

            yp8 = inp.tile([PARTS, KT, E, ITEM], f8)
            yt8 = inp.tile([PARTS, KT, E, ITEM], f8)
            nc.gpsimd.dma_start(out=yp8, in_=ypv)
            nc.gpsimd.dma_start(out=yt8, in_=ytv)

            M2 = work.tile([PARTS, KT, AW], f16)
            nc.vector.memset(M2[:, :, COL_ONE : COL_ONE + 1], 1.0)

            # ---- smooth l1 columns (latency-first order on DVE) ----
            dd = work.tile([PARTS, KT, E, P], f16)
            nc.vector.tensor_sub(dd, yp8[:, :, :, C:ITEM], yt8[:, :, :, C:ITEM])
            w1v = M2[:, :, COL_W1 : COL_W1 + NW].rearrange(
                "p k (e j) -> p k e j", j=P
            )
            nc.vector.tensor_scalar(
                out=w1v, in0=dd, scalar1=1.0, scalar2=0.0,
                op0=OP.subtract, op1=OP.max,
            )
            w2v = M2[:, :, COL_W2 : COL_W2 + NW].rearrange(
                "p k (e j) -> p k e j", j=P
            )
            nc.vector.tensor_scalar(
                out=w2v, in0=dd, scalar1=1.0, scalar2=0.0,
                op0=OP.add, op1=OP.min,
            )

            # ---- cross entropy: lse = ln(sum_c exp(z)) ----
            ex = work.tile([PARTS, KT, E, C], f16)
            nc.scalar.activation(out=ex, in_=yp8[:, :, :, 0:C], func=AF.Exp)
            vv = work.tile([PARTS, KT, E, P], f16)
            nc.scalar.activation(out=vv, in_=dd, func=AF.Square)
            h1 = work.tile([PARTS, KT, E, 8], f16)
            nc.vector.tensor_tensor(h1, ex[:, :, :, 0:8], ex[:, :, :, 8:16], OP.add)
            h2 = work.tile([PARTS, KT, E, 4], f16)
            nc.vector.tensor_tensor(h2, h1[:, :, :, 0:4], h1[:, :, :, 4:8], OP.add)
            h3 = work.tile([PARTS, KT, E, 2], f16)
            nc.vector.tensor_tensor(h3, h2[:, :, :, 0:2], h2[:, :, :, 2:4], OP.add)
            s = work.tile([PARTS, KT, E], f16)
            nc.vector.tensor_tensor(s, h3[:, :, :, 0], h3[:, :, :, 1], OP.add)
            mmv = M2[:, :, COL_MM : COL_MM + NW].rearrange(
                "p k (e j) -> p k e j", j=P
            )
            nc.vector.tensor_scalar(
                out=mmv, in0=vv, scalar1=1.0, scalar2=None, op0=OP.min
            )
            nc.scalar.activation(
                out=M2[:, :, COL_LSE : COL_LSE + E], in_=s, func=AF.Ln
            )

            # ---- gram accumulation on PE (mm2 first: needs only the DMAs) ----
            ytf = yt8.rearrange("p k e i -> p k (e i)")
            for k in range(KT):
                first = ci == 0 and k == 0
                last = ci == nchunks - 1 and k == KT - 1
                nc.tensor.matmul(
                    psB, ytf[:, k], yp8[:, k, :, 0:C], start=first, stop=last
                )
            for k in range(KT):
                first = ci == 0 and k == 0
                last = ci == nchunks - 1 and k == KT - 1
                nc.tensor.matmul(
                    psA, ytf[:, k], M2[:, k], start=first, stop=last
                )

        stage = singles.tile([D, GW], f32)
        nc.scalar.copy(stage[:, 0:AW], psA)
        nc.scalar.copy(stage[:, AW:GW], psB)
        nc.sync.dma_start(out=out_ab[:], in_=stage)

    # Pre-load the one ACT table set covering Exp/Ln/Square/Copy
    # (natural_log_exp_and_others); avoids per-activation table thrash.
    from concourse.hw_specs import get_activation_tables

    tables = list(get_activation_tables(nc.m.arch).items())
    set_id = next(
        i for i, (name, _) in enumerate(tables)
        if name == "natural_log_exp_and_others"
    )
    load = mybir.InstLoadActFuncSet(
        name=nc.get_next_instruction_name(), act_func_set_id=set_id, ins=[], outs=[]
    )
    load.engine = mybir.EngineType.Activation
    nc.register_instruction(load)
    placed = False
    for blk in nc.m.functions[0].blocks:
        for idx, inst in enumerate(blk.instructions):
            if isinstance(inst, mybir.InstActivation):
                blk.instructions.insert(idx, load)
                placed = True
                break
        if placed:
            break
    assert placed

    nc.compile()
    return nc


def _get_nc():
    if "nc" not in _CACHE:
        _CACHE["nc"] = _build_bass()
    return _CACHE["nc"]


def kernel(y_pred, y_true, num_params_per_effect):
    from concourse.bass_utils import run_bass_kernel_spmd

    yp = np.ascontiguousarray(np.asarray(y_pred, dtype=np.float32))
    yt = np.ascontiguousarray(np.asarray(y_true, dtype=np.float32))
    npf = np.asarray(num_params_per_effect, dtype=np.int64)

    yp_sh = yp.reshape(NCORES, N_CORE, D)
    yt_sh = yt.reshape(NCORES, N_CORE, D)
    in_maps = [{"y_pred": yp_sh[i], "y_true": yt_sh[i]} for i in range(NCORES)]

    nc = _get_nc()
    results = run_bass_kernel_spmd(nc, in_maps, list(range(NCORES))).results

    # ---- host-side scalar assembly in float64 ----
    G = np.zeros((D, GW), np.float64)
    for res in results:
        G += np.asarray(res["out_ab"], np.float64)
    GA = G[:, 0:AW]
    GB = G[:, AW:GW]

    Tmask = (np.arange(P)[None, :] < npf[:, None]).astype(np.float64)  # [C,P]
    MSUM = 0.0
    PCNT = 0.0
    LSEt = 0.0
    DX = 0.0
    AFSX = 0.0
    RSUM = 0.0
    for e in range(E):
        rows = slice(ITEM * e, ITEM * e + C)  # yt logit rows of slot e
        cnt = GA[rows, COL_ONE]  # per-class active counts [C]
        MSUM += cnt.sum()
        PCNT += (npf * cnt).sum()
        LSEt += GA[rows, COL_LSE + e].sum()
        DX += np.trace(GB[rows, C * e : C * e + C])
        AFSX += GB[rows, C * e : C * e + C].sum()
        cols = slice(P * e, P * (e + 1))
        sl1 = (
            GA[rows, COL_W1 + P * e : COL_W1 + P * (e + 1)]
            - GA[rows, COL_W2 + P * e : COL_W2 + P * (e + 1)]
            + 0.5 * GA[rows, COL_MM + P * e : COL_MM + P * (e + 1)]
        )
        RSUM += (Tmask * sl1).sum()

    CSUM = LSEt - (1.0 - LS) * DX - (LS / C) * AFSX

    loss_cls = CSUM / max(MSUM, 1.0) if MSUM > 0 else 0.0
    # num_params_per_effect >= 1 guarantees PCNT >= MSUM, so the
    # reference's unmasked-reg fallback (psum==0 while msum>0) is dead.
    loss_reg = (RSUM / max(PCNT, 1.0) if PCNT > 0 else 0.0) if MSUM > 0 else 0.0
    total = loss_cls + REG_W * loss_reg

    return (
        np.float32(total),
        np.float32(loss_cls),
        np.float32(loss_reg),
    )
